# revision 8
# baseline (speedup 1.0000x reference)
"""Trainium2 Bass kernel: 3x3 conv (NCHW 32x256x56x56, 256->256ch, pad 1) with
a host-expanded synthesized weight, data-parallel over 8 NeuronCores.

Conv is computed as implicit GEMM: for each of the 9 kernel taps, a matmul
over a zero-padded (58x58) input image held in SBUF with input channels on
partitions, accumulating in PSUM.  fp32r (FP22 multiply, fp32 accumulate)
runs the PE at full rate for moving free-dim >= 256; we use N = 8 rows x 56
cols = 448.
"""

import numpy as np

# Problem constants (hardcoded per contract; kernel.py must be self-contained)
OOC, OIC, K1, K2 = 64, 64, 3, 3
R0, R1 = 4, 4
N_CORES = 8
BATCH = 32
N_PER_CORE = BATCH // N_CORES  # 4
C = 256
H = W = 56
HP = WP = H + 2  # zero-padded spatial
RB = 8           # output rows per matmul chunk -> N = RB*W = 448
NCH = H // RB    # 7 chunks
KT = C // 128    # 2 input-channel tiles
MT = C // 128    # 2 output-channel tiles
POS = K1 * K2    # 9 kernel taps

_NC_CACHE = {}
LAST_RESULT = {}  # test.py introspection: last BassKernelResults


def _expand_weight(weight, alphas, betas):
    """W[p0*64+i, p1*64+j, ky, kx] = w[i,j,ky,kx] * a[p0,p1] / (1+exp(w*b[p0,p1]))."""
    w = weight.astype(np.float32)[None, None]            # (1,1,64,64,3,3)
    a = alphas.astype(np.float32).reshape(R0, R1)[:, :, None, None, None, None]
    b = betas.astype(np.float32).reshape(R0, R1)[:, :, None, None, None, None]
    act = w * a / (1.0 + np.exp(w * b))                  # (4,4,64,64,3,3)
    return act.transpose(0, 2, 1, 3, 4, 5).reshape(R0 * OOC, R1 * OIC, K1, K2)


def _build_nc():
    import concourse.mybir as mybir
    import concourse.tile as tile
    from concourse import bacc

    fp32 = mybir.dt.float32
    fp32r = mybir.dt.float32r

    nc = bacc.Bacc("TRN2", target_bir_lowering=False, debug=False,
                   num_devices=N_CORES)

    x_d = nc.dram_tensor("x", [N_PER_CORE, C, H, W], fp32r, kind="ExternalInput")
    w_d = nc.dram_tensor("w", [128, KT, MT, POS, 128], fp32r, kind="ExternalInput")
    b_d = nc.dram_tensor("b", [128, MT], fp32, kind="ExternalInput")
    o_d = nc.dram_tensor("out", [N_PER_CORE, C, H, W], fp32, kind="ExternalOutput")

    with tile.TileContext(nc) as tc:
        with (
            tc.tile_pool(name="const", bufs=1) as const_pool,
            tc.tile_pool(name="xpad", bufs=1) as xp_pool,
            tc.tile_pool(name="ot", bufs=4) as out_pool,
            tc.tile_pool(name="ps", bufs=6, space="PSUM") as psum_pool,
        ):
            w_sb = const_pool.tile([128, KT, MT, POS, 128], fp32r,
                                   name="w_sb", tag="w_sb")
            nc.sync.dma_start(w_sb[:], w_d.ap())
            b_sb = const_pool.tile([128, MT], fp32, name="b_sb", tag="b_sb")
            nc.sync.dma_start(b_sb[:], b_d.ap())

            # Double-buffered padded input images; pad ring zeroed once.
            xp = [[xp_pool.tile([128, HP, WP], fp32r, name=f"xp{par}_{kt}",
                                tag=f"xp{par}_{kt}")
                   for kt in range(KT)] for par in range(2)]
            for par in range(2):
                for kt in range(KT):
                    t = xp[par][kt]
                    nc.vector.memset(t[:, 0:1, :].bitcast(fp32), 0.0)
                    nc.vector.memset(t[:, HP - 1:HP, :].bitcast(fp32), 0.0)
                    nc.vector.memset(t[:, 1:HP - 1, 0:1].bitcast(fp32), 0.0)
                    nc.vector.memset(t[:, 1:HP - 1, WP - 1:WP].bitcast(fp32), 0.0)

            xap = x_d.ap()
            oap = o_d.ap()
            for n in range(N_PER_CORE):
                par = n % 2
                for kt in range(KT):
                    nc.sync.dma_start(
                        xp[par][kt][:, 1:H + 1, 1:W + 1],
                        xap[n, kt * 128:(kt + 1) * 128, :, :])
                for ch in range(NCH):
                    y0 = ch * RB
                    for mt in range(MT):
                        ps = psum_pool.tile([128, RB, W], fp32,
                                            name="ps", tag="ps")
                        first = True
                        for kt in range(KT):
                            for dy in range(K1):
                                for dx in range(K2):
                                    pos = dy * K2 + dx
                                    last = (kt == KT - 1 and pos == POS - 1)
                                    nc.tensor.matmul(
                                        ps[:, :, :],
                                        w_sb[:, kt, mt, pos, :],
                                        xp[par][kt][:, y0 + dy:y0 + dy + RB,
                                                    dx:dx + W],
                                        start=first, stop=last,
                                    )
                                    first = False
                        ot = out_pool.tile([128, RB, W], fp32,
                                           name="ot", tag="ot")
                        nc.vector.tensor_scalar_add(ot[:], ps[:],
                                                    b_sb[:, mt:mt + 1])
                        nc.sync.dma_start(
                            oap[n, mt * 128:(mt + 1) * 128, y0:y0 + RB, :],
                            ot[:])
    nc.compile()
    return nc


def get_nc():
    if "nc" not in _NC_CACHE:
        _NC_CACHE["nc"] = _build_nc()
    return _NC_CACHE["nc"]


def kernel(x, weight, alphas, betas, bias):
    from concourse.bass_utils import run_bass_kernel_spmd

    x = np.ascontiguousarray(np.asarray(x, dtype=np.float32))
    Wfull = _expand_weight(np.asarray(weight), np.asarray(alphas),
                           np.asarray(betas))  # (256,256,3,3)

    # lhsT layout: [ci_local(128 partitions), kt, mt, pos, co_local(128)]
    Wt = Wfull.transpose(1, 0, 2, 3).reshape(C, C, POS)       # (ci, co, pos)
    w_arr = np.ascontiguousarray(
        Wt.reshape(KT, 128, MT, 128, POS).transpose(1, 0, 2, 4, 3))
    b_arr = np.ascontiguousarray(
        np.asarray(bias, dtype=np.float32).reshape(MT, 128).T)

    nc = get_nc()
    in_maps = [
        {"x": x[i * N_PER_CORE:(i + 1) * N_PER_CORE], "w": w_arr, "b": b_arr}
        for i in range(N_CORES)
    ]
    res = run_bass_kernel_spmd(nc, in_maps, core_ids=list(range(N_CORES)))
    LAST_RESULT["res"] = res
    return np.concatenate([r["out"] for r in res.results], axis=0)
